# revision 9
# baseline (speedup 1.0000x reference)
"""Trainium2 Bass kernel for nn_DeformConv2d_72765335929324.

The module is a dense 3x3 conv (stride 1, pad 1) [B,64,256,256] -> [B,18,256,256]
plus a per-pixel additive `offset` term and a channel bias.

Strategy (per core; batch is sharded 2 images/core across 8 cores):
- conv = 9 taps, each a [cin=64 -> cout=18] matmul over shifted input views.
- PE array packing via tile_position: 2 images on row-groups {0,64} x 4
  output row-pair chunks on col-groups {0,32,64,96} -> 8 concurrent matmul
  streams, 9 accumulating taps each, N=512 (2 output rows) per stream.
- offset+bias are pre-added on the host; on device a single full-width
  [128,512] DVE add folds PSUM + offset -> SBUF, then DMA out.
- W-edge padding handled by shrinking the matmul N-span per kw tap;
  H-edge padding by shrinking the row span of the first/last chunk taps.
"""

import os
import numpy as np

import concourse.bass as bass
import concourse.tile as tile
import concourse.mybir as mybir
from concourse.vector_clock import ScopedClock
from concourse.bass_utils import run_bass_kernel_spmd

B, CIN, H, W = 16, 64, 256, 256
COUT = 18
COUTP = 32  # cout padded to a 32-wide PE column group
NCORES = 8
BPC = B // NCORES  # images per core

# tap order: full-coverage center tap first so start=True initializes the
# whole PSUM bank region before partial-coverage taps accumulate.
TAPS = [(1, 1), (0, 0), (0, 1), (0, 2), (1, 0), (1, 2), (2, 0), (2, 1), (2, 2)]

# kw -> (src col offset, dst col offset, ncols): zero-pad at W edges is
# realized by shrinking the span instead of padding SBUF.
KW_SPAN = {0: (0, 1, W - 1), 1: (0, 0, W), 2: (1, 0, W - 1)}

DT_NAME = os.environ.get("CONV_DT", "float32r")
R = 32  # output rows per input slab


class _TileContext(tile.TileContext):
    """TileContext whose tail drain spreads its semaphore waits over NOPs.

    The stock _drain_and_barrier puts one wait per logical proc on a single
    Drain instruction; the walrus build here rejects instructions carrying
    more than 2 sync waits.
    """

    def _drain_and_barrier(self, tick_clock, wait_clock):
        nc = self.nc
        carriers = [nc.sync.nop(nofuse=True) for _ in range(64)]
        drain_inst = nc.sync.drain()
        wait_clock.add_sem_waits(
            drain_inst.ins, ScopedClock({None: tick_clock.global_clock})
        )
        si = drain_inst.ins.sync_info
        waits = list(si.on_wait or [])
        if len(waits) > 1:
            si.on_wait = waits[:1]
            extra = waits[1:]
            assert len(extra) <= len(carriers)
            for wt, nop in zip(extra, carriers):
                nsi = nop.ins.sync_info
                if nsi is None:
                    nop.ins.sync_info = mybir.SyncInfo(on_wait=[wt], on_update=[])
                else:
                    nsi.on_wait = [wt]
        nc.all_engine_barrier()
        assert self.sems is not None
        popped = nc._tile_sem_poison_stack.pop()
        assert popped is self._sem_poison
        nc.clear_and_free_semaphores(list(self.sems.allocated().values()))
        nc.all_engine_barrier()


def _split_excess_waits(nc):
    """Spill per-instruction semaphore waits onto same-engine NOP carriers.

    Tile's wait assigner attaches up to ~6 waits to one instruction; the
    walrus build here rejects >1 sync wait on engine instructions (>2 on
    EventSemaphore). DMA descriptors are left untouched. A NOP that runs
    just before the instruction on the same engine is semantically
    equivalent (program order on one engine is serial).
    """
    for bb in nc.m.functions[0].blocks:
        new = []
        for inst in bb.instructions:
            si = inst.sync_info
            waits = list(si.on_wait) if si and si.on_wait else []
            if isinstance(inst, mybir.InstEventSemaphore):
                cap = 2
            else:
                cap = 1
            if len(waits) > cap:
                si.on_wait = waits[:cap]
                for w in waits[cap:]:
                    n = mybir.InstNoOp(
                        name=nc.get_next_instruction_name(), ins=[], outs=[]
                    )
                    n.engine = inst.engine
                    n.sync_info = mybir.SyncInfo(on_wait=[w], on_update=[])
                    new.append(n)
            new.append(inst)
        bb.instructions = new


def build_nc(dt_name=DT_NAME, h=H):
    dt_in = getattr(mybir.dt, dt_name)
    f32 = mybir.dt.float32
    nc = bass.Bass()
    x = nc.dram_tensor("x", [BPC, CIN, h, W], dt_in, kind="ExternalInput")
    off = nc.dram_tensor("off", [BPC, COUTP, h, W], f32, kind="ExternalInput")
    wt = nc.dram_tensor("w", [128, len(TAPS) * COUTP], dt_in, kind="ExternalInput")
    y = nc.dram_tensor("y", [BPC, COUT, h, W], f32, kind="ExternalOutput")

    nslab = h // R
    with _TileContext(nc) as tc:
        with (
            tc.tile_pool(name="wpool", bufs=1) as wpool,
            tc.tile_pool(name="slabp", bufs=2) as slabp,
            tc.tile_pool(name="offp", bufs=6) as offp,
            tc.tile_pool(name="outp", bufs=6) as outp,
            tc.tile_pool(name="psump", bufs=4, space="PSUM") as psump,
        ):
            w_t = wpool.tile([128, len(TAPS) * COUTP], dt_in, name="w_t")
            nc.sync.dma_start(w_t[:, :], wt[:, :])

            for s in range(nslab):
                # slab slot j <-> input row s*R - 1 + j (34 slots incl. halo)
                slab = slabp.tile([128, (R + 2) * W], dt_in, name="slab")
                r_lo = max(0, s * R - 1)
                r_hi = min(h, s * R + R + 1)
                slot0 = r_lo - (s * R - 1)
                for img in range(BPC):
                    nc.sync.dma_start(
                        slab[
                            img * 64 : (img + 1) * 64,
                            slot0 * W : (slot0 + (r_hi - r_lo)) * W,
                        ],
                        x[img, :, r_lo:r_hi, :],
                    )
                slab3 = [
                    slab[img * 64 : (img + 1) * 64, :].rearrange(
                        "p (r w) -> p r w", w=W
                    )
                    for img in range(BPC)
                ]

                for q in range(4):
                    offts, psums = [], []
                    for img in range(BPC):
                        off_t = offp.tile([128, 2 * W], f32, name="off_t")
                        for c in range(4):
                            gr0 = s * R + q * 8 + 2 * c
                            nc.sync.dma_start(
                                off_t[32 * c : 32 * (c + 1), :],
                                off[img, :, gr0 : gr0 + 2, :],
                            )
                        psum_t = psump.tile([128, 2 * W], f32, name="psum_t")
                        offts.append(off_t)
                        psums.append(psum_t)

                    # t-major emission: 8 streams (4 col-groups x 2 images)
                    # advance through the taps in lockstep in the PE array.
                    for ti, (kh, kw) in enumerate(TAPS):
                        for c in range(4):
                            for img in range(BPC):
                                r0 = q * 8 + 2 * c
                                gr0 = s * R + r0
                                row_lo, nrows = 0, 2
                                if gr0 == 0 and kh == 0:
                                    row_lo, nrows = 1, 1
                                if gr0 == h - 2 and kh == 2:
                                    nrows = 1
                                src_off, dst_off, ncol = KW_SPAN[kw]
                                slot = r0 + row_lo + kh
                                rhs = slab3[img][
                                    :, slot : slot + nrows, src_off : src_off + ncol
                                ]
                                out_ap = psums[img][
                                    32 * c : 32 * c + COUTP, :
                                ].rearrange("p (r w) -> p r w", w=W)[
                                    :, row_lo : row_lo + nrows, dst_off : dst_off + ncol
                                ]
                                lhsT = w_t[
                                    img * 64 : (img + 1) * 64,
                                    ti * COUTP : (ti + 1) * COUTP,
                                ]
                                nc.tensor.matmul(
                                    out_ap,
                                    lhsT,
                                    rhs,
                                    start=(ti == 0),
                                    stop=(ti == len(TAPS) - 1),
                                    tile_position=(img * 64, 32 * c),
                                    # the sim's accumulation-group sanity check
                                    # mis-addresses partition-sliced PSUM groups;
                                    # its per-element pending-zero modeling is
                                    # partition-aware and stays active.
                                    skip_group_check=True,
                                )

                    for img in range(BPC):
                        out_t = outp.tile([128, 2 * W], f32, name="out_t")
                        nc.vector.tensor_add(out_t[:, :], psums[img][:, :], offts[img][:, :])
                        for c in range(4):
                            gr0 = s * R + q * 8 + 2 * c
                            nc.sync.dma_start(
                                y[img, :, gr0 : gr0 + 2, :],
                                out_t[32 * c : 32 * c + COUT, :],
                            )
    _split_excess_waits(nc)
    return nc


def pack_inputs(input, offset, weight, bias, dt_name=DT_NAME, h=H):
    np_in = mybir.dt.np(getattr(mybir.dt, dt_name))
    input = np.asarray(input, dtype=np.float32)
    offset = np.asarray(offset, dtype=np.float32)
    weight = np.asarray(weight, dtype=np.float32)
    bias = np.asarray(bias, dtype=np.float32)

    offb = np.zeros((B, COUTP) + offset.shape[2:], dtype=np.float32)
    offb[:, :COUT] = offset[:, :COUT] + bias[None, :, None, None]
    w_packed = np.zeros((128, len(TAPS) * COUTP), dtype=np_in)
    for t, (kh, kw) in enumerate(TAPS):
        w_packed[0:64, t * COUTP : t * COUTP + COUT] = weight[:, :, kh, kw].T.astype(
            np_in
        )
    w_packed[64:128] = w_packed[0:64]
    xc = input.astype(np_in)
    in_maps = [
        {
            "x": np.ascontiguousarray(xc[BPC * k : BPC * (k + 1), :, :h]),
            "off": np.ascontiguousarray(offb[BPC * k : BPC * (k + 1), :, :h]),
            "w": w_packed,
        }
        for k in range(NCORES)
    ]
    return in_maps


_NC_CACHE = {}


def run_on_hw(input, offset, weight, bias, dt_name=DT_NAME, trace=False):
    key = dt_name
    if key not in _NC_CACHE:
        _NC_CACHE[key] = build_nc(dt_name)
    nc = _NC_CACHE[key]
    in_maps = pack_inputs(input, offset, weight, bias, dt_name)
    res = run_bass_kernel_spmd(nc, in_maps, list(range(NCORES)), trace=trace)
    out = np.concatenate([res.results[k]["y"] for k in range(NCORES)], axis=0)
    return out.astype(np.float32, copy=False), res


def kernel(input, offset, weight, bias):
    out, _ = run_on_hw(input, offset, weight, bias)
    return out
